# revision 8
# baseline (speedup 1.0000x reference)
"""Trainium2 Bass kernel for hashgrid encode + MLP + SH (nn_Hash1SH).

Contract: kernel(**inputs) takes FULL unsharded inputs, returns FULL output.
Sharding: data-parallel over points (8 cores x 32768 points), tables
replicated (host-interleaved so one gather row serves both tables).
"""
import numpy as np

import concourse.bass as bass
import concourse.bacc as bacc
import concourse.mybir as mybir
from concourse.tile import TileContext
from concourse.bass_utils import run_bass_kernel_spmd
from concourse.masks import make_identity

# ---- problem constants (hardcoded; kernel.py must be self-contained) ----
N = 262144
NCORES = 8
NLOC = N // NCORES          # 32768 points per core
P = 128
C = NLOC // P               # 256 columns
L = 16
F = 2
T = 1 << 19
M19 = T - 1
BASE, DESIRED = 16, 512
_SCALE = np.exp(np.log(DESIRED / BASE) / (L - 1))
RES = np.floor(BASE * _SCALE ** np.arange(L)).astype(np.float32)
PRIMES = (1, 2654435761, 805459861)
P1_19 = PRIMES[1] & M19
P2_19 = PRIMES[2] & M19
W = 32
SH_C0 = 0.28209479177387814
SH_C1 = 0.4886025119029199
SH_C2 = [1.0925484305920792, -1.0925484305920792, 0.31539156525252005,
         -1.0925484305920792, 0.5462742152960396]
SH_C3 = [-0.5900435899266435, 2.890611442640554, -0.4570457994644658,
         0.3731763325901154, -0.4570457994644658, 1.445305721320277,
         -0.5900435899266435]

f32 = mybir.dt.float32
i32 = mybir.dt.int32
Alu = mybir.AluOpType
Act = mybir.ActivationFunctionType

_NC_CACHE = {}


def _bc(ap, n):
    """Broadcast an AP by appending a step-0 dim of size n."""
    return bass.AP(ap.tensor, ap.offset, list(ap.ap) + [[0, n]])


def build_nc():
    nc = bacc.Bacc(None)
    xs_d = nc.dram_tensor("xs", [NLOC, 3], f32, kind="ExternalInput")
    ds_d = nc.dram_tensor("ds", [NLOC, 3], f32, kind="ExternalInput")
    emb_d = nc.dram_tensor("emb", [L * T, 4], f32, kind="ExternalInput")
    mlpw_d = nc.dram_tensor("mlpw", [W, 6 * W], f32, kind="ExternalInput")
    # mlpw columns: [lw1 | lw2 | lw3(pad to 32) | ww1 | ww2 | ww3(pad 32)]
    mlpb_d = nc.dram_tensor("mlpb", [W, 6], f32, kind="ExternalInput")
    out_d = nc.dram_tensor("outc", [NLOC, 3], f32, kind="ExternalOutput")

    with TileContext(nc) as tc:
        with tc.tile_pool(name="persist", bufs=1) as pp:
            ident = pp.tile([P, P], f32)
            make_identity(nc, ident[:])
            xst = pp.tile([P, C, 3], f32)
            dst = pp.tile([P, C, 3], f32)
            nc.sync.dma_start(out=xst[:], in_=xs_d[:].rearrange("(p c) d -> p c d", p=P))
            nc.sync.dma_start(out=dst[:], in_=ds_d[:].rearrange("(p c) d -> p c d", p=P))
            wt = pp.tile([W, 6 * W], f32)
            bt = pp.tile([W, 6], f32)
            nc.sync.dma_start(out=wt[:], in_=mlpw_d[:])
            nc.sync.dma_start(out=bt[:], in_=mlpb_d[:])
            accx = pp.tile([P, C, 2 * L], f32)
            accw = pp.tile([P, C, 2 * L], f32)
            # int constant tiles
            c_one = pp.tile([P, 1], i32, tag="c_one")
            c_m19 = pp.tile([P, 1], i32, tag="c_m19")
            c_511 = pp.tile([P, 1], i32, tag="c_511")
            c_10 = pp.tile([P, 1], i32, tag="c_10")
            c_p1 = pp.tile([P, 1], i32, tag="c_p1")
            c_p2 = pp.tile([P, 1], i32, tag="c_p2")
            nc.gpsimd.memset(c_one[:], 1)
            nc.gpsimd.memset(c_m19[:], M19)
            nc.gpsimd.memset(c_511[:], 511)
            nc.gpsimd.memset(c_10[:], 10)
            nc.gpsimd.memset(c_p1[:], P1_19)
            nc.gpsimd.memset(c_p2[:], P2_19)

            with tc.tile_pool(name="lvl", bufs=1) as lp:
                for lvl in range(L):
                    res = float(RES[lvl])
                    h0 = [None] * 3  # int hash term for lower corner, per dim
                    h1 = [None] * 3
                    w0 = [None] * 3  # 1-frac
                    w1 = [None] * 3  # frac
                    for d in range(3):
                        xcomp = xst[:, :, d]  # [P, C] strided
                        posm = lp.tile([P, C], f32, tag=f"posm{d}")
                        nc.vector.tensor_scalar(out=posm[:], in0=xcomp, scalar1=res,
                                                scalar2=0.5 * res - 0.5,
                                                op0=Alu.mult, op1=Alu.add)
                        i0 = lp.tile([P, C], i32, tag=f"i0{d}")
                        nc.vector.tensor_copy(out=i0[:], in_=posm[:])  # rint
                        f0 = lp.tile([P, C], f32, tag=f"f0{d}")
                        nc.vector.tensor_copy(out=f0[:], in_=i0[:])
                        fr = lp.tile([P, C], f32, tag=f"fr{d}")
                        # frac = posm - (f0 - 0.5)
                        fm = lp.tile([P, C], f32, tag=f"fm{d}")
                        nc.vector.tensor_scalar(out=fm[:], in0=f0[:], scalar1=-0.5,
                                                scalar2=None, op0=Alu.add)
                        nc.vector.tensor_tensor(out=fr[:], in0=posm[:], in1=fm[:],
                                                op=Alu.subtract)
                        w1[d] = fr
                        om = lp.tile([P, C], f32, tag=f"om{d}")
                        nc.vector.tensor_scalar(out=om[:], in0=fr[:], scalar1=-1.0,
                                                scalar2=1.0, op0=Alu.mult, op1=Alu.add)
                        w0[d] = om
                        if d == 0:
                            h0[d] = i0
                            hx1 = lp.tile([P, C], i32, tag="hx1")
                            nc.vector.tensor_tensor(out=hx1[:], in0=i0[:],
                                                    in1=_bc(c_one[:, 0:1], C),
                                                    op=Alu.add)
                            h1[d] = hx1
                        else:
                            pc = P1_19 if d == 1 else P2_19
                            cpt = c_p1 if d == 1 else c_p2
                            bhi = float(pc >> 10)
                            clo = float(pc & 1023)
                            yb = lp.tile([P, C], f32, tag=f"yb{d}")
                            nc.vector.tensor_scalar(out=yb[:], in0=f0[:], scalar1=bhi,
                                                    scalar2=None, op0=Alu.mult)
                            yc = lp.tile([P, C], f32, tag=f"yc{d}")
                            nc.vector.tensor_scalar(out=yc[:], in0=f0[:], scalar1=clo,
                                                    scalar2=None, op0=Alu.mult)
                            iyb = lp.tile([P, C], i32, tag=f"iyb{d}")
                            nc.vector.tensor_copy(out=iyb[:], in_=yb[:])
                            nc.vector.tensor_tensor(out=iyb[:], in0=iyb[:],
                                                    in1=_bc(c_511[:, 0:1], C),
                                                    op=Alu.bitwise_and)
                            nc.vector.tensor_tensor(out=iyb[:], in0=iyb[:],
                                                    in1=_bc(c_10[:, 0:1], C),
                                                    op=Alu.logical_shift_left)
                            iyc = lp.tile([P, C], i32, tag=f"iyc{d}")
                            nc.vector.tensor_copy(out=iyc[:], in_=yc[:])
                            hh0 = lp.tile([P, C], i32, tag=f"hh0{d}")
                            nc.vector.tensor_tensor(out=hh0[:], in0=iyb[:], in1=iyc[:],
                                                    op=Alu.add)
                            nc.vector.tensor_tensor(out=hh0[:], in0=hh0[:],
                                                    in1=_bc(c_m19[:, 0:1], C),
                                                    op=Alu.bitwise_and)
                            h0[d] = hh0
                            hh1 = lp.tile([P, C], i32, tag=f"hh1{d}")
                            nc.vector.tensor_tensor(out=hh1[:], in0=hh0[:],
                                                    in1=_bc(cpt[:, 0:1], C),
                                                    op=Alu.add)
                            nc.vector.tensor_tensor(out=hh1[:], in0=hh1[:],
                                                    in1=_bc(c_m19[:, 0:1], C),
                                                    op=Alu.bitwise_and)
                            h1[d] = hh1

                    # corner hashes and weights; corner k = (a<<2)|(b<<1)|cz
                    hyz = []
                    wyz = []
                    for b in range(2):
                        for cz in range(2):
                            hy = h1[1] if b else h0[1]
                            hz = h1[2] if cz else h0[2]
                            t_h = lp.tile([P, C], i32, tag=f"hyz{b}{cz}")
                            nc.vector.tensor_tensor(out=t_h[:], in0=hy[:], in1=hz[:],
                                                    op=Alu.bitwise_xor)
                            hyz.append(t_h)
                            wy = w1[1] if b else w0[1]
                            wz = w1[2] if cz else w0[2]
                            t_w = lp.tile([P, C], f32, tag=f"wyz{b}{cz}")
                            nc.vector.tensor_tensor(out=t_w[:], in0=wy[:], in1=wz[:],
                                                    op=Alu.mult)
                            wyz.append(t_w)
                    idx_k = []
                    w_k = []
                    for a in range(2):
                        hx = h1[0] if a else h0[0]
                        wx = w1[0] if a else w0[0]
                        for j in range(4):
                            t_i = lp.tile([P, C], i32, tag=f"idx{a}{j}")
                            nc.vector.tensor_tensor(out=t_i[:], in0=hx[:],
                                                    in1=hyz[j][:], op=Alu.bitwise_xor)
                            idx_k.append(t_i)
                            t_w = lp.tile([P, C], f32, tag=f"wk{a}{j}")
                            nc.vector.tensor_tensor(out=t_w[:], in0=wx[:],
                                                    in1=wyz[j][:], op=Alu.mult)
                            w_k.append(t_w)

                    feats = [lp.tile([P, C, 4], f32, name=f"feat{k}", tag=f"feat{k}") for k in range(8)]
                    for cc in range(C):
                        for k in range(8):
                            nc.gpsimd.indirect_dma_start(
                                out=feats[k][:, cc, :],
                                out_offset=None,
                                in_=emb_d[:],
                                in_offset=bass.IndirectOffsetOnAxis(
                                    ap=idx_k[k][:, cc:cc + 1], axis=0),
                                element_offset=lvl * T * 4,
                            )

                    accl = lp.tile([P, C, 4], f32, tag="accl")
                    tmp = lp.tile([P, C, 4], f32, tag="tmpm")
                    nc.vector.tensor_tensor(out=accl[:], in0=feats[0][:],
                                            in1=_bc(w_k[0][:], 4), op=Alu.mult)
                    for k in range(1, 8):
                        nc.vector.tensor_tensor(out=tmp[:], in0=feats[k][:],
                                                in1=_bc(w_k[k][:], 4), op=Alu.mult)
                        nc.vector.tensor_tensor(out=accl[:], in0=accl[:], in1=tmp[:],
                                                op=Alu.add)
                    nc.vector.tensor_copy(out=accx[:, :, 2 * lvl:2 * lvl + 2],
                                          in_=accl[:, :, 0:2])
                    nc.vector.tensor_copy(out=accw[:, :, 2 * lvl:2 * lvl + 2],
                                          in_=accl[:, :, 2:4])

            # ---------------- MLP + SH phase ----------------
            with tc.tile_pool(name="mlp", bufs=1) as mp, \
                 tc.tile_pool(name="ps", bufs=1, space="PSUM") as psp:
                sh_pm = mp.tile([P, C, 16], f32)
                ws_pm = mp.tile([P, C, 30], f32)
                NCHUNK = 8
                CC = C // NCHUNK          # 32 cols per chunk
                NPT = CC * P              # 4096 points per chunk
                for ch in range(NCHUNK):
                    for tbl in range(2):
                        acc = accx if tbl == 0 else accw
                        xT = mp.tile([W, NPT], f32, tag="xT")
                        for s in range(CC):
                            pt = psp.tile([W, P], f32, tag="ptr")
                            nc.tensor.transpose(out=pt[:],
                                                in_=acc[:, ch * CC + s, :],
                                                identity=ident[:])
                            nc.vector.tensor_copy(out=xT[:, s * P:(s + 1) * P],
                                                  in_=pt[:])
                        wofs = 0 if tbl == 0 else 3 * W
                        h1t = mp.tile([W, NPT], f32, tag="h1t")
                        h2t = mp.tile([W, NPT], f32, tag="h2t")
                        NO = 16 if tbl == 0 else 30
                        o3t = mp.tile([W, NPT], f32, tag="o3t")
                        for sub in range(NPT // 512):
                            sl = slice(sub * 512, (sub + 1) * 512)
                            ps1 = psp.tile([W, 512], f32, tag="ps1")
                            nc.tensor.matmul(ps1[:], lhsT=wt[:, wofs:wofs + W],
                                             rhs=xT[:, sl], start=True, stop=True)
                            nc.scalar.activation(h1t[:, sl], ps1[:], Act.Relu,
                                                 bias=bt[:, 3 * tbl:3 * tbl + 1])
                            ps2 = psp.tile([W, 512], f32, tag="ps2")
                            nc.tensor.matmul(ps2[:], lhsT=wt[:, wofs + W:wofs + 2 * W],
                                             rhs=h1t[:, sl], start=True, stop=True)
                            nc.scalar.activation(h2t[:, sl], ps2[:], Act.Relu,
                                                 bias=bt[:, 3 * tbl + 1:3 * tbl + 2])
                            ps3 = psp.tile([W, 512], f32, tag="ps3")
                            nc.tensor.matmul(ps3[:], lhsT=wt[:, wofs + 2 * W:wofs + 3 * W],
                                             rhs=h2t[:, sl], start=True, stop=True)
                            nc.scalar.activation(o3t[:, sl], ps3[:], Act.Identity,
                                                 bias=bt[:, 3 * tbl + 2:3 * tbl + 3])
                        dstt = sh_pm if tbl == 0 else ws_pm
                        for s in range(CC):
                            ptb = psp.tile([P, W], f32, tag="ptb")
                            nc.tensor.transpose(out=ptb[:, :NO],
                                                in_=o3t[:NO, s * P:(s + 1) * P],
                                                identity=ident[:NO, :NO])
                            nc.vector.tensor_copy(out=dstt[:, ch * CC + s, :],
                                                  in_=ptb[:, :NO])

                # ---- SH eval + final tiny matmuls (points-major, wide) ----
                def tt(o, a, b_, op):
                    nc.vector.tensor_tensor(out=o, in0=a, in1=b_, op=op)

                tA = mp.tile([P, C], f32, tag="tA")
                tB = mp.tile([P, C], f32, tag="tB")
                dx = mp.tile([P, C], f32, tag="dx")
                dy = mp.tile([P, C], f32, tag="dy")
                dz = mp.tile([P, C], f32, tag="dz")
                # normalize d
                r2 = mp.tile([P, C], f32, tag="r2")
                tt(r2[:], dst[:, :, 0], dst[:, :, 0], Alu.mult)
                tt(tA[:], dst[:, :, 1], dst[:, :, 1], Alu.mult)
                tt(r2[:], r2[:], tA[:], Alu.add)
                tt(tA[:], dst[:, :, 2], dst[:, :, 2], Alu.mult)
                tt(r2[:], r2[:], tA[:], Alu.add)
                inv = mp.tile([P, C], f32, tag="inv")
                nc.vector.reciprocal(out=inv[:], in_=r2[:])
                sc = mp.tile([P, C], f32, tag="sc")
                nc.scalar.activation(sc[:], inv[:], Act.Sqrt)
                tt(dx[:], dst[:, :, 0], sc[:], Alu.mult)
                tt(dy[:], dst[:, :, 1], sc[:], Alu.mult)
                tt(dz[:], dst[:, :, 2], sc[:], Alu.mult)

                xx = mp.tile([P, C], f32, tag="xx")
                yy = mp.tile([P, C], f32, tag="yy")
                zz = mp.tile([P, C], f32, tag="zz")
                xy = mp.tile([P, C], f32, tag="xy")
                yz = mp.tile([P, C], f32, tag="yz")
                xz = mp.tile([P, C], f32, tag="xz")
                tt(xx[:], dx[:], dx[:], Alu.mult)
                tt(yy[:], dy[:], dy[:], Alu.mult)
                tt(zz[:], dz[:], dz[:], Alu.mult)
                tt(xy[:], dx[:], dy[:], Alu.mult)
                tt(yz[:], dy[:], dz[:], Alu.mult)
                tt(xz[:], dx[:], dz[:], Alu.mult)

                cres = mp.tile([P, C], f32, tag="cres")

                def addterm(coef_ap_or_none, k, scalar_coef):
                    """cres += scalar_coef * basis * sh_pm[..k]; basis in tA (or None=1)."""
                    if coef_ap_or_none is None:
                        nc.vector.tensor_scalar(out=tB[:], in0=sh_pm[:, :, k],
                                                scalar1=scalar_coef, scalar2=None,
                                                op0=Alu.mult)
                    else:
                        tt(tB[:], coef_ap_or_none, sh_pm[:, :, k], Alu.mult)
                        nc.vector.tensor_scalar(out=tB[:], in0=tB[:],
                                                scalar1=scalar_coef, scalar2=None,
                                                op0=Alu.mult)
                    tt(cres[:], cres[:], tB[:], Alu.add)

                nc.vector.tensor_scalar(out=cres[:], in0=sh_pm[:, :, 0],
                                        scalar1=SH_C0, scalar2=None, op0=Alu.mult)
                addterm(dy[:], 1, -SH_C1)
                addterm(dz[:], 2, SH_C1)
                addterm(dx[:], 3, -SH_C1)
                addterm(xy[:], 4, SH_C2[0])
                addterm(yz[:], 5, SH_C2[1])
                # C2[2]*(2zz-xx-yy)
                nc.vector.tensor_scalar(out=tA[:], in0=zz[:], scalar1=2.0,
                                        scalar2=None, op0=Alu.mult)
                tt(tA[:], tA[:], xx[:], Alu.subtract)
                tt(tA[:], tA[:], yy[:], Alu.subtract)
                addterm(tA[:], 6, SH_C2[2])
                addterm(xz[:], 7, SH_C2[3])
                xmy = mp.tile([P, C], f32, tag="xmy")
                tt(xmy[:], xx[:], yy[:], Alu.subtract)
                addterm(xmy[:], 8, SH_C2[4])
                # C3 terms
                nc.vector.tensor_scalar(out=tA[:], in0=xx[:], scalar1=3.0,
                                        scalar2=None, op0=Alu.mult)
                tt(tA[:], tA[:], yy[:], Alu.subtract)
                tt(tA[:], tA[:], dy[:], Alu.mult)
                addterm(tA[:], 9, SH_C3[0])
                tt(tA[:], xy[:], dz[:], Alu.mult)
                addterm(tA[:], 10, SH_C3[1])
                nc.vector.tensor_scalar(out=tA[:], in0=zz[:], scalar1=4.0,
                                        scalar2=None, op0=Alu.mult)
                tt(tA[:], tA[:], xx[:], Alu.subtract)
                tt(tA[:], tA[:], yy[:], Alu.subtract)
                ttmp = mp.tile([P, C], f32, tag="ttmp")
                nc.vector.tensor_copy(out=ttmp[:], in_=tA[:])
                tt(tA[:], tA[:], dy[:], Alu.mult)
                addterm(tA[:], 11, SH_C3[2])
                # C3[3]*z*(2zz-3xx-3yy)
                nc.vector.tensor_scalar(out=tA[:], in0=zz[:], scalar1=2.0,
                                        scalar2=None, op0=Alu.mult)
                nc.vector.tensor_scalar(out=tB[:], in0=xx[:], scalar1=3.0,
                                        scalar2=None, op0=Alu.mult)
                tt(tA[:], tA[:], tB[:], Alu.subtract)
                nc.vector.tensor_scalar(out=tB[:], in0=yy[:], scalar1=3.0,
                                        scalar2=None, op0=Alu.mult)
                tt(tA[:], tA[:], tB[:], Alu.subtract)
                tt(tA[:], tA[:], dz[:], Alu.mult)
                addterm(tA[:], 12, SH_C3[3])
                tt(tA[:], ttmp[:], dx[:], Alu.mult)
                addterm(tA[:], 13, SH_C3[4])
                tt(tA[:], xmy[:], dz[:], Alu.mult)
                addterm(tA[:], 14, SH_C3[5])
                tt(tA[:], xmy[:], dx[:], Alu.mult)
                addterm(tA[:], 15, SH_C3[6])

                # final: c1_j = relu(cres*m1_j + b1_j)  (m1=ws[0:3], b1=ws[3:6])
                c1 = [mp.tile([P, C], f32, name=f"c1_{j}", tag=f"c1_{j}") for j in range(3)]
                for j in range(3):
                    tt(c1[j][:], cres[:], ws_pm[:, :, j], Alu.mult)
                    tt(c1[j][:], c1[j][:], ws_pm[:, :, 3 + j], Alu.add)
                    nc.vector.tensor_scalar(out=c1[j][:], in0=c1[j][:], scalar1=0.0,
                                            scalar2=None, op0=Alu.max)
                # c2_j = relu(sum_s c1_s*m2[s,j] + b2_j)  m2 at 6+s*3+j, b2 at 15+j
                c2 = [mp.tile([P, C], f32, name=f"c2_{j}", tag=f"c2_{j}") for j in range(3)]
                for j in range(3):
                    tt(c2[j][:], c1[0][:], ws_pm[:, :, 6 + j], Alu.mult)
                    for s in range(1, 3):
                        tt(tB[:], c1[s][:], ws_pm[:, :, 6 + s * 3 + j], Alu.mult)
                        tt(c2[j][:], c2[j][:], tB[:], Alu.add)
                    tt(c2[j][:], c2[j][:], ws_pm[:, :, 15 + j], Alu.add)
                    nc.vector.tensor_scalar(out=c2[j][:], in0=c2[j][:], scalar1=0.0,
                                            scalar2=None, op0=Alu.max)
                # c3_j = sigmoid(sum_s c2_s*m3[s,j] + b3_j)  m3 at 18+s*3+j, b3 27+j
                outt = mp.tile([P, C, 3], f32, tag="outt")
                for j in range(3):
                    tt(tA[:], c2[0][:], ws_pm[:, :, 18 + j], Alu.mult)
                    for s in range(1, 3):
                        tt(tB[:], c2[s][:], ws_pm[:, :, 18 + s * 3 + j], Alu.mult)
                        tt(tA[:], tA[:], tB[:], Alu.add)
                    tt(tA[:], tA[:], ws_pm[:, :, 27 + j], Alu.add)
                    nc.scalar.activation(outt[:, :, j], tA[:], Act.Sigmoid)

                nc.sync.dma_start(out=out_d[:].rearrange("(p c) d -> p c d", p=P),
                                  in_=outt[:])
    nc.compile()
    return nc


def kernel(xs, ds, emb_x, emb_w, lw1, lb1, lw2, lb2, lw3, lb3,
           ww1, wb1, ww2, wb2, ww3, wb3):
    xs = np.asarray(xs, dtype=np.float32)
    ds = np.asarray(ds, dtype=np.float32)
    emb_il = np.concatenate(
        [np.asarray(emb_x, np.float32).reshape(L * T, F),
         np.asarray(emb_w, np.float32).reshape(L * T, F)], axis=1)  # [L*T, 4]
    mlpw = np.zeros((W, 6 * W), np.float32)
    mlpw[:, 0:W] = lw1
    mlpw[:, W:2 * W] = lw2
    mlpw[:, 2 * W:2 * W + 16] = lw3
    mlpw[:, 3 * W:4 * W] = ww1
    mlpw[:, 4 * W:5 * W] = ww2
    mlpw[:, 5 * W:5 * W + 30] = ww3
    mlpb = np.zeros((W, 6), np.float32)
    mlpb[:, 0] = lb1
    mlpb[:, 1] = lb2
    mlpb[:16, 2] = lb3
    mlpb[:, 3] = wb1
    mlpb[:, 4] = wb2
    mlpb[:30, 5] = wb3

    if "nc" not in _NC_CACHE:
        _NC_CACHE["nc"] = build_nc()
    nc = _NC_CACHE["nc"]

    in_maps = []
    for r in range(NCORES):
        sl = slice(r * NLOC, (r + 1) * NLOC)
        in_maps.append({"xs": np.ascontiguousarray(xs[sl]),
                        "ds": np.ascontiguousarray(ds[sl]),
                        "emb": emb_il, "mlpw": mlpw, "mlpb": mlpb})
    res = run_bass_kernel_spmd(nc, in_maps, list(range(NCORES))).results
    return np.concatenate([res[r]["outc"] for r in range(NCORES)], axis=0)



# revision 9
# speedup vs baseline: 1.9343x; 1.9343x over previous
"""Trainium2 Bass kernel for hashgrid encode + MLP + SH (nn_Hash1SH) — v3.

Gather strategy: the per-[P,1] indirect DMA costs ~1.2us/instruction (128
descriptors) and the SWDGE descriptor path sustains ~9ns/descriptor, so the
baseline's 32768 tiny gathers are descriptor/instruction bound.  v3 uses the
custom multi-index `dma_gather` ucode: each instruction gathers 8192 pages of
256B (16 table rows) by int16 page id, optionally spread over multiple SWDGE
queues; the needed 16B row is then selected on-chip with a mask + strided
tree reduction fused with the trilinear weighting.

HW-decoded dma_gather semantics (probed on trn2):
  request g reads idx16[16 + g%16, 8*(g//128) + (g%128)//16]  (partitions
  16..31!), element g lands at out[g%128, g//128, :].  The interpreter reads
  partitions 0..15 with the same formula, so we write indices at partitions
  0..15 (partition-aligned DVE copies) and copy 0..15 -> 16..31 with one
  small SBUF->SBUF DMA (DMA can cross partitions; DVE cannot).

Layouts:
  request g = j*128 + p, j = c*8 + k (c = col within 8-col chunk, k corner)
  feats [P, 64, 64]f32 per chunk; selection+weighting yields acc64
  [P, C, 64]bf16 (feature dim = level*4 + {x0,x1,w0,w1}) feeding one fused
  dual-MLP (both tables' 3 layers as 64-wide block matrices, bf16 matmuls).
"""
import numpy as np
from ml_dtypes import bfloat16

import concourse.bass as bass
import concourse.bacc as bacc
import concourse.mybir as mybir
from concourse.tile import TileContext
from concourse.bass_utils import run_bass_kernel_spmd
from concourse.masks import make_identity

N = 262144
NCORES = 8
NLOC = N // NCORES          # 32768 points per core
P = 128
C = NLOC // P               # 256 columns
CC = 8                      # cols per gather chunk
NCH = C // CC               # 32 chunks per level
NIDX = 8 * CC * P           # 8192 gather requests per chunk
S16 = NIDX // 16            # 512 idx positions per partition
NQ = 2                      # SWDGE queues for dma_gather
L = 16
F = 2
T = 1 << 19
M19 = T - 1
NPAGE = T // 16             # 32768 256B pages per level
BASE, DESIRED = 16, 512
_SCALE = np.exp(np.log(DESIRED / BASE) / (L - 1))
RES = np.floor(BASE * _SCALE ** np.arange(L)).astype(np.float32)
PRIMES = (1, 2654435761, 805459861)
P1_19 = PRIMES[1] & M19
P2_19 = PRIMES[2] & M19
W = 32
W2 = 2 * W
NO3 = 16 + 30
SH_C0 = 0.28209479177387814
SH_C1 = 0.4886025119029199
SH_C2 = [1.0925484305920792, -1.0925484305920792, 0.31539156525252005,
         -1.0925484305920792, 0.5462742152960396]
SH_C3 = [-0.5900435899266435, 2.890611442640554, -0.4570457994644658,
         0.3731763325901154, -0.4570457994644658, 1.445305721320277,
         -0.5900435899266435]

f32 = mybir.dt.float32
bf16 = mybir.dt.bfloat16
i32 = mybir.dt.int32
i16 = mybir.dt.int16
Alu = mybir.AluOpType
Act = mybir.ActivationFunctionType

_NC_CACHE = {}
_RUN_OPTS = {"trace": False, "trace_cores": None}
_LAST = {}


def _bc(ap, n):
    return bass.AP(ap.tensor, ap.offset, list(ap.ap) + [[0, n]])


def _hash_level(nc, lp, xst, lvl, idx8, w8, page_f, within, consts):
    """Hash + trilinear weights for one level over all C cols.

    Outputs ([P, C, 8] c-major, corner-minor):
      w8      f32  trilinear corner weights
      page_f  f32  page id = idx >> 4  (exact, <= 32767)
      within  bf16 row within page = idx & 15 (exact)
    idx8: i32 scratch, destroyed.
    """
    c_one, c_m19, c_511, c_10, c_p1, c_p2, c_4, c_15 = consts
    res = float(RES[lvl])
    h0 = [None] * 3
    h1 = [None] * 3
    w0 = [None] * 3
    w1 = [None] * 3
    for d in range(3):
        xcomp = xst[:, :, d]
        posm = lp.tile([P, C], f32, tag="posm")
        nc.vector.tensor_scalar(out=posm[:], in0=xcomp, scalar1=res,
                                scalar2=0.5 * res - 0.5,
                                op0=Alu.mult, op1=Alu.add)
        i0 = lp.tile([P, C], i32, tag="i0x" if d == 0 else "i0s")
        nc.vector.tensor_copy(out=i0[:], in_=posm[:])  # rint
        f0 = lp.tile([P, C], f32, tag="f0")
        nc.vector.tensor_copy(out=f0[:], in_=i0[:])
        fr = lp.tile([P, C], f32, tag=f"fr{d}")
        fm = lp.tile([P, C], f32, tag="fm")
        nc.vector.tensor_scalar(out=fm[:], in0=f0[:], scalar1=-0.5,
                                scalar2=None, op0=Alu.add)
        nc.vector.tensor_tensor(out=fr[:], in0=posm[:], in1=fm[:],
                                op=Alu.subtract)
        w1[d] = fr
        om = lp.tile([P, C], f32, tag=f"om{d}")
        nc.vector.tensor_scalar(out=om[:], in0=fr[:], scalar1=-1.0,
                                scalar2=1.0, op0=Alu.mult, op1=Alu.add)
        w0[d] = om
        if d == 0:
            h0[d] = i0
            hx1 = lp.tile([P, C], i32, tag="hx1")
            nc.vector.tensor_tensor(out=hx1[:], in0=i0[:],
                                    in1=_bc(c_one[:, 0:1], C), op=Alu.add)
            h1[d] = hx1
        else:
            pc = P1_19 if d == 1 else P2_19
            cpt = c_p1 if d == 1 else c_p2
            bhi = float(pc >> 10)
            clo = float(pc & 1023)
            yb = lp.tile([P, C], f32, tag="yb")
            nc.vector.tensor_scalar(out=yb[:], in0=f0[:], scalar1=bhi,
                                    scalar2=None, op0=Alu.mult)
            yc = lp.tile([P, C], f32, tag="yc")
            nc.vector.tensor_scalar(out=yc[:], in0=f0[:], scalar1=clo,
                                    scalar2=None, op0=Alu.mult)
            iyb = lp.tile([P, C], i32, tag="iyb")
            nc.vector.tensor_copy(out=iyb[:], in_=yb[:])
            nc.vector.tensor_tensor(out=iyb[:], in0=iyb[:],
                                    in1=_bc(c_511[:, 0:1], C),
                                    op=Alu.bitwise_and)
            nc.vector.tensor_tensor(out=iyb[:], in0=iyb[:],
                                    in1=_bc(c_10[:, 0:1], C),
                                    op=Alu.logical_shift_left)
            iyc = lp.tile([P, C], i32, tag="iyc")
            nc.vector.tensor_copy(out=iyc[:], in_=yc[:])
            hh0 = lp.tile([P, C], i32, tag=f"hh0{d}")
            nc.vector.tensor_tensor(out=hh0[:], in0=iyb[:], in1=iyc[:],
                                    op=Alu.add)
            nc.vector.tensor_tensor(out=hh0[:], in0=hh0[:],
                                    in1=_bc(c_m19[:, 0:1], C),
                                    op=Alu.bitwise_and)
            h0[d] = hh0
            hh1 = lp.tile([P, C], i32, tag=f"hh1{d}")
            nc.vector.tensor_tensor(out=hh1[:], in0=hh0[:],
                                    in1=_bc(cpt[:, 0:1], C), op=Alu.add)
            nc.vector.tensor_tensor(out=hh1[:], in0=hh1[:],
                                    in1=_bc(c_m19[:, 0:1], C),
                                    op=Alu.bitwise_and)
            h1[d] = hh1

    hyz = []
    wyz = []
    for b in range(2):
        for cz in range(2):
            hy = h1[1] if b else h0[1]
            hz = h1[2] if cz else h0[2]
            t_h = lp.tile([P, C], i32, tag=f"hyz{b}{cz}")
            nc.vector.tensor_tensor(out=t_h[:], in0=hy[:], in1=hz[:],
                                    op=Alu.bitwise_xor)
            hyz.append(t_h)
            wy = w1[1] if b else w0[1]
            wz = w1[2] if cz else w0[2]
            t_w = lp.tile([P, C], f32, tag=f"wyz{b}{cz}")
            nc.vector.tensor_tensor(out=t_w[:], in0=wy[:], in1=wz[:],
                                    op=Alu.mult)
            wyz.append(t_w)
    for a in range(2):
        hx = h1[0] if a else h0[0]
        wx = w1[0] if a else w0[0]
        for j in range(4):
            k = a * 4 + j
            nc.vector.tensor_tensor(out=idx8[:, :, k], in0=hx[:],
                                    in1=hyz[j][:], op=Alu.bitwise_xor)
            nc.vector.tensor_tensor(out=w8[:, :, k], in0=wx[:],
                                    in1=wyz[j][:], op=Alu.mult)
    # batched page/within prep on [P, C*8]; idx8 destroyed (in-place shift)
    iscr = lp.tile([P, C, 8], i32, tag="iscr")
    nc.vector.tensor_tensor(out=iscr[:], in0=idx8[:],
                            in1=_bc(_bc(c_15[:, 0:1], C), 8),
                            op=Alu.bitwise_and)
    nc.vector.tensor_copy(out=within[:], in_=iscr[:])
    nc.vector.tensor_tensor(out=idx8[:], in0=idx8[:],
                            in1=_bc(_bc(c_4[:, 0:1], C), 8),
                            op=Alu.logical_shift_right)
    nc.vector.tensor_copy(out=page_f[:], in_=idx8[:])


def build_nc(sim_dup=False):
    del sim_dup  # idx data is always present at partitions 0..15 too
    nc = bacc.Bacc(None, num_swdge_queues=NQ)
    xs_d = nc.dram_tensor("xs", [NLOC, 3], f32, kind="ExternalInput")
    ds_d = nc.dram_tensor("ds", [NLOC, 3], f32, kind="ExternalInput")
    embl_d = [nc.dram_tensor(f"embl{l}", [NPAGE, 64], f32, kind="ExternalInput")
              for l in range(L)]
    mlpw_d = nc.dram_tensor("mlpw", [W2, 2 * W2 + NO3], bf16, kind="ExternalInput")
    mlpb_d = nc.dram_tensor("mlpb", [W2, 3], f32, kind="ExternalInput")
    out_d = nc.dram_tensor("outc", [NLOC, 3], f32, kind="ExternalOutput")

    with TileContext(nc) as tc:
        with tc.tile_pool(name="persist", bufs=1) as pp:
            ident = pp.tile([P, P], f32)
            make_identity(nc, ident[:])
            identb = pp.tile([P, P], bf16)
            nc.vector.tensor_copy(out=identb[:], in_=ident[:])
            xst = pp.tile([P, C, 3], f32)
            dst = pp.tile([P, C, 3], f32)
            nc.sync.dma_start(out=xst[:], in_=xs_d[:].rearrange("(p c) d -> p c d", p=P))
            nc.sync.dma_start(out=dst[:], in_=ds_d[:].rearrange("(p c) d -> p c d", p=P))
            wt = pp.tile([W2, 2 * W2 + NO3], bf16)
            bt = pp.tile([W2, 3], f32)
            nc.sync.dma_start(out=wt[:], in_=mlpw_d[:])
            nc.sync.dma_start(out=bt[:], in_=mlpb_d[:])
            sh_pm = pp.tile([P, C, 16], f32)
            ws_pm = pp.tile([P, C, 30], f32)
            acc64 = pp.tile([P, C, 64], bf16)
            cvals = [1, M19, 511, 10, P1_19, P2_19, 4, 15]
            cts = []
            for i, v in enumerate(cvals):
                ct = pp.tile([P, 1], i32, name=f"c_{i}", tag=f"c_{i}")
                nc.gpsimd.memset(ct[:], v)
                cts.append(ct)
            consts = tuple(cts)
            iota16 = pp.tile([P, 16], bf16, tag="iota16")
            iota16i = pp.tile([P, 16], i32, tag="iota16i")
            nc.gpsimd.iota(iota16i[:], pattern=[[1, 16]], base=0,
                           channel_multiplier=0)
            nc.vector.tensor_copy(out=iota16[:], in_=iota16i[:])

            # ping-pong idx16 tiles (written at partitions 0..15, DMA-shifted
            # to 16..31; tails zeroed once for the interpreter's range check)
            idx16s = []
            for i in range(2):
                t = pp.tile([P, 2 * S16], i16, name=f"idx16_{i}", tag=f"idx16_{i}")
                for p0 in (32, 64, 96):
                    nc.vector.memset(t[p0:p0 + 32, :], 0)
                idx16s.append(t)

            with tc.tile_pool(name="lvl", bufs=1) as lp, \
                 tc.tile_pool(name="ch", bufs=2) as cp, \
                 tc.tile_pool(name="ch1", bufs=1) as cp1, \
                 tc.tile_pool(name="mm", bufs=2) as mp, \
                 tc.tile_pool(name="psA", bufs=2, space="PSUM") as psA, \
                 tc.tile_pool(name="psB", bufs=1, space="PSUM") as psB:

                for lvl in range(L):
                    idx8 = lp.tile([P, C, 8], i32, tag="idx8")
                    w8 = lp.tile([P, C, 8], f32, tag="w8")
                    page_f = lp.tile([P, C, 8], f32, tag="page_f")
                    within = lp.tile([P, C, 8], bf16, tag="within")
                    _hash_level(nc, lp, xst, lvl, idx8, w8, page_f, within,
                                consts)
                    for chp in range(NCH // 2):   # chunk pairs
                        idx16 = idx16s[chp % 2]
                        for h in range(2):
                            ch = chp * 2 + h
                            # --- idx16 build (partition-aligned writes) ---
                            pf_sl = page_f[:, ch * CC:(ch + 1) * CC, :]
                            t1p = psA.tile([CC * 8, P], f32, tag="t1p")
                            nc.tensor.transpose(
                                out=t1p[:],
                                in_=bass.AP(pf_sl.tensor, pf_sl.offset,
                                            [pf_sl.ap[0], [1, CC * 8]]),
                                identity=ident[:])
                            t1c = cp.tile([CC * 8, P], f32, tag="t1c")
                            nc.vector.tensor_copy(out=t1c[:], in_=t1p[:])
                            for b in range(8):
                                t2p = psA.tile([16, CC * 8], f32, tag="t1p")
                                nc.tensor.transpose(
                                    out=t2p[:],
                                    in_=t1c[:, b * 16:(b + 1) * 16],
                                    identity=ident[:CC * 8, :CC * 8])
                                lo = idx16[0:16, :]
                                nc.vector.tensor_copy(
                                    out=bass.AP(lo.tensor,
                                                lo.offset + h * S16 + b,
                                                [list(lo.ap)[0], [8, CC * 8]]),
                                    in_=t2p[:])
                        # partition shift 0..15 -> 16..31 (both chunks at once)
                        nc.sync.dma_start(out=idx16[16:32, :],
                                          in_=idx16[0:16, :])
                        for h in range(2):
                            ch = chp * 2 + h
                            feats = cp.tile([P, 64, 64], f32, tag="feats")
                            nc.gpsimd.dma_gather(
                                out_ap=feats[:], in_ap=embl_d[lvl][:],
                                idxs_ap=idx16[:, h * S16:(h + 1) * S16],
                                num_idxs=NIDX, num_idxs_reg=NIDX,
                                elem_size=64, single_packet=False,
                                queue_num=(chp * 2 + h) % NQ)
                            # --- selection: cmp then masked tree-reduce ---
                            cmp = cp1.tile([P, 64, 16], f32, tag="cmp")
                            win_sl = within[:, ch * CC:(ch + 1) * CC, :]
                            win_ap = bass.AP(win_sl.tensor, win_sl.offset,
                                             [win_sl.ap[0], [1, 64], [0, 16]])
                            ifull = iota16[:]
                            iota_ap = bass.AP(ifull.tensor, ifull.offset,
                                              [list(ifull.ap)[0], [0, 64],
                                               [1, 16]])
                            nc.vector.tensor_tensor(out=cmp[:], in0=win_ap,
                                                    in1=iota_ap,
                                                    op=Alu.is_equal)
                            fv = feats[:]  # [P, 64, 64] = [P, j, 16, 4]
                            cmp_ap = bass.AP(cmp[:].tensor, cmp[:].offset,
                                             [cmp[:].ap[0], [16, 64], [1, 16],
                                              [0, 4]])
                            f4 = bass.AP(fv.tensor, fv.offset,
                                         [fv.ap[0], [64, 64], [4, 16], [1, 4]])
                            nc.vector.tensor_tensor(out=f4, in0=f4, in1=cmp_ap,
                                                    op=Alu.mult)

                            def fsl(b0, n):
                                return bass.AP(fv.tensor, fv.offset + b0 * 4,
                                               [fv.ap[0], [64, 64], [1, n * 4]])
                            nc.vector.tensor_tensor(out=fsl(0, 8), in0=fsl(0, 8),
                                                    in1=fsl(8, 8), op=Alu.add)
                            nc.vector.tensor_tensor(out=fsl(0, 4), in0=fsl(0, 4),
                                                    in1=fsl(4, 4), op=Alu.add)
                            nc.vector.tensor_tensor(out=fsl(0, 2), in0=fsl(0, 2),
                                                    in1=fsl(2, 2), op=Alu.add)
                            nc.vector.tensor_tensor(out=fsl(0, 1), in0=fsl(0, 1),
                                                    in1=fsl(1, 1), op=Alu.add)
                            # --- trilinear weight + corner sum + acc write ---
                            sel = bass.AP(fv.tensor, fv.offset,
                                          [fv.ap[0], [64, 64], [1, 4]])
                            w_sl = w8[:, ch * CC:(ch + 1) * CC, :]
                            w_ap = bass.AP(w_sl.tensor, w_sl.offset,
                                           [w_sl.ap[0], [1, 64], [0, 4]])
                            wsel = cp1.tile([P, 64, 4], f32, tag="wsel")
                            nc.vector.tensor_tensor(out=wsel[:], in0=sel,
                                                    in1=w_ap, op=Alu.mult)

                            def wsl(k0, n):
                                return bass.AP(wsel[:].tensor,
                                               wsel[:].offset + k0 * 4,
                                               [wsel[:].ap[0], [32, CC],
                                                [1, n * 4]])
                            nc.vector.tensor_tensor(out=wsl(0, 4), in0=wsl(0, 4),
                                                    in1=wsl(4, 4), op=Alu.add)
                            nc.vector.tensor_tensor(out=wsl(0, 2), in0=wsl(0, 2),
                                                    in1=wsl(2, 2), op=Alu.add)
                            nc.vector.tensor_tensor(out=wsl(0, 1), in0=wsl(0, 1),
                                                    in1=wsl(1, 1), op=Alu.add)
                            acc_sl = acc64[:, ch * CC:(ch + 1) * CC,
                                           4 * lvl:4 * lvl + 4]
                            nc.vector.tensor_copy(out=acc_sl, in_=wsl(0, 1))

                # ---------------- fused dual-MLP ----------------
                for s in range(C // 4):        # 64 subs of 512 points
                    xT = mp.tile([W2, 512], bf16, tag="xT")
                    for c4 in range(4):
                        col = s * 4 + c4
                        ptf = psB.tile([W2, P], bf16, tag="ptf")
                        nc.tensor.transpose(out=ptf[:], in_=acc64[:, col, :],
                                            identity=identb[:])
                        nc.vector.tensor_copy(out=xT[:, c4 * P:(c4 + 1) * P],
                                              in_=ptf[:])
                    ps1 = psB.tile([W2, 512], f32, tag="ps1")
                    nc.tensor.matmul(ps1[:], lhsT=wt[:, 0:W2], rhs=xT[:],
                                     start=True, stop=True)
                    h1t = mp.tile([W2, 512], bf16, tag="h1t")
                    nc.scalar.activation(h1t[:], ps1[:], Act.Relu,
                                         bias=bt[:, 0:1])
                    ps2 = psB.tile([W2, 512], f32, tag="ps2")
                    nc.tensor.matmul(ps2[:], lhsT=wt[:, W2:2 * W2], rhs=h1t[:],
                                     start=True, stop=True)
                    h2t = mp.tile([W2, 512], bf16, tag="h2t")
                    nc.scalar.activation(h2t[:], ps2[:], Act.Relu,
                                         bias=bt[:, 1:2])
                    ps3 = psB.tile([NO3, 512], f32, tag="ps3")
                    nc.tensor.matmul(ps3[:], lhsT=wt[:, 2 * W2:2 * W2 + NO3],
                                     rhs=h2t[:], start=True, stop=True)
                    o3t = mp.tile([NO3, 512], f32, tag="o3t")
                    nc.scalar.activation(o3t[:], ps3[:], Act.Identity,
                                         bias=bt[:NO3, 2:3])
                    for c4 in range(4):
                        col = s * 4 + c4
                        ptb = psB.tile([P, NO3], f32, tag="ptb")
                        nc.tensor.transpose(out=ptb[:],
                                            in_=o3t[:, c4 * P:(c4 + 1) * P],
                                            identity=ident[:NO3, :NO3])
                        nc.vector.tensor_copy(out=sh_pm[:, col, :],
                                              in_=ptb[:, 0:16])
                        nc.vector.tensor_copy(out=ws_pm[:, col, :],
                                              in_=ptb[:, 16:NO3])

                # ---------------- SH eval + final tiny matmuls --------------
                # direction basis (tags shared with dead hash temps)
                def tt(o, a, b_, op):
                    nc.vector.tensor_tensor(out=o, in0=a, in1=b_, op=op)

                def ltile(tag):
                    return lp.tile([P, C], f32, name=f"ph3_{tag}", tag=tag)

                dx = ltile("posm")
                dy = ltile("f0")
                dz = ltile("fm")
                xx = ltile("fr0")
                yy = ltile("fr1")
                zz = ltile("fr2")
                xy = ltile("om0")
                yz = ltile("om1")
                xz = ltile("om2")
                r2 = ltile("yb")
                sc = ltile("yc")
                tt(r2[:], dst[:, :, 0], dst[:, :, 0], Alu.mult)
                tt(sc[:], dst[:, :, 1], dst[:, :, 1], Alu.mult)
                tt(r2[:], r2[:], sc[:], Alu.add)
                tt(sc[:], dst[:, :, 2], dst[:, :, 2], Alu.mult)
                tt(r2[:], r2[:], sc[:], Alu.add)
                inv = ltile("hx1")
                nc.vector.reciprocal(out=inv[:], in_=r2[:])
                nc.scalar.activation(sc[:], inv[:], Act.Sqrt)
                tt(dx[:], dst[:, :, 0], sc[:], Alu.mult)
                tt(dy[:], dst[:, :, 1], sc[:], Alu.mult)
                tt(dz[:], dst[:, :, 2], sc[:], Alu.mult)
                tt(xx[:], dx[:], dx[:], Alu.mult)
                tt(yy[:], dy[:], dy[:], Alu.mult)
                tt(zz[:], dz[:], dz[:], Alu.mult)
                tt(xy[:], dx[:], dy[:], Alu.mult)
                tt(yz[:], dy[:], dz[:], Alu.mult)
                tt(xz[:], dx[:], dz[:], Alu.mult)

                tA = ltile("hyz00")
                tB = ltile("hyz01")
                cres = ltile("hyz10")
                xmy = ltile("hyz11")
                ttmp = ltile("wyz00")

                def addterm(coef_ap_or_none, k, scalar_coef):
                    if coef_ap_or_none is None:
                        nc.vector.tensor_scalar(out=tB[:], in0=sh_pm[:, :, k],
                                                scalar1=scalar_coef,
                                                scalar2=None, op0=Alu.mult)
                    else:
                        tt(tB[:], coef_ap_or_none, sh_pm[:, :, k], Alu.mult)
                        nc.vector.tensor_scalar(out=tB[:], in0=tB[:],
                                                scalar1=scalar_coef,
                                                scalar2=None, op0=Alu.mult)
                    tt(cres[:], cres[:], tB[:], Alu.add)

                nc.vector.tensor_scalar(out=cres[:], in0=sh_pm[:, :, 0],
                                        scalar1=SH_C0, scalar2=None,
                                        op0=Alu.mult)
                addterm(dy[:], 1, -SH_C1)
                addterm(dz[:], 2, SH_C1)
                addterm(dx[:], 3, -SH_C1)
                addterm(xy[:], 4, SH_C2[0])
                addterm(yz[:], 5, SH_C2[1])
                nc.vector.tensor_scalar(out=tA[:], in0=zz[:], scalar1=2.0,
                                        scalar2=None, op0=Alu.mult)
                tt(tA[:], tA[:], xx[:], Alu.subtract)
                tt(tA[:], tA[:], yy[:], Alu.subtract)
                addterm(tA[:], 6, SH_C2[2])
                addterm(xz[:], 7, SH_C2[3])
                tt(xmy[:], xx[:], yy[:], Alu.subtract)
                addterm(xmy[:], 8, SH_C2[4])
                nc.vector.tensor_scalar(out=tA[:], in0=xx[:], scalar1=3.0,
                                        scalar2=None, op0=Alu.mult)
                tt(tA[:], tA[:], yy[:], Alu.subtract)
                tt(tA[:], tA[:], dy[:], Alu.mult)
                addterm(tA[:], 9, SH_C3[0])
                tt(tA[:], xy[:], dz[:], Alu.mult)
                addterm(tA[:], 10, SH_C3[1])
                nc.vector.tensor_scalar(out=tA[:], in0=zz[:], scalar1=4.0,
                                        scalar2=None, op0=Alu.mult)
                tt(tA[:], tA[:], xx[:], Alu.subtract)
                tt(tA[:], tA[:], yy[:], Alu.subtract)
                nc.vector.tensor_copy(out=ttmp[:], in_=tA[:])
                tt(tA[:], tA[:], dy[:], Alu.mult)
                addterm(tA[:], 11, SH_C3[2])
                nc.vector.tensor_scalar(out=tA[:], in0=zz[:], scalar1=2.0,
                                        scalar2=None, op0=Alu.mult)
                nc.vector.tensor_scalar(out=tB[:], in0=xx[:], scalar1=3.0,
                                        scalar2=None, op0=Alu.mult)
                tt(tA[:], tA[:], tB[:], Alu.subtract)
                nc.vector.tensor_scalar(out=tB[:], in0=yy[:], scalar1=3.0,
                                        scalar2=None, op0=Alu.mult)
                tt(tA[:], tA[:], tB[:], Alu.subtract)
                tt(tA[:], tA[:], dz[:], Alu.mult)
                addterm(tA[:], 12, SH_C3[3])
                tt(tA[:], ttmp[:], dx[:], Alu.mult)
                addterm(tA[:], 13, SH_C3[4])
                tt(tA[:], xmy[:], dz[:], Alu.mult)
                addterm(tA[:], 14, SH_C3[5])
                tt(tA[:], xmy[:], dx[:], Alu.mult)
                addterm(tA[:], 15, SH_C3[6])

                c1 = [ltile(f"wyz{b}{cz}") for b, cz in ((0, 1), (1, 0), (1, 1))]
                for j in range(3):
                    tt(c1[j][:], cres[:], ws_pm[:, :, j], Alu.mult)
                    tt(c1[j][:], c1[j][:], ws_pm[:, :, 3 + j], Alu.add)
                    nc.vector.tensor_scalar(out=c1[j][:], in0=c1[j][:],
                                            scalar1=0.0, scalar2=None,
                                            op0=Alu.max)
                c2 = [ltile(t) for t in ("hh02", "hh12", "iyb")]
                for j in range(3):
                    tt(c2[j][:], c1[0][:], ws_pm[:, :, 6 + j], Alu.mult)
                    for s in range(1, 3):
                        tt(tB[:], c1[s][:], ws_pm[:, :, 6 + s * 3 + j], Alu.mult)
                        tt(c2[j][:], c2[j][:], tB[:], Alu.add)
                    tt(c2[j][:], c2[j][:], ws_pm[:, :, 15 + j], Alu.add)
                    nc.vector.tensor_scalar(out=c2[j][:], in0=c2[j][:],
                                            scalar1=0.0, scalar2=None,
                                            op0=Alu.max)
                outt = lp.tile([P, C, 3], f32, tag="outt")
                for j in range(3):
                    tt(tA[:], c2[0][:], ws_pm[:, :, 18 + j], Alu.mult)
                    for s in range(1, 3):
                        tt(tB[:], c2[s][:], ws_pm[:, :, 18 + s * 3 + j], Alu.mult)
                        tt(tA[:], tA[:], tB[:], Alu.add)
                    tt(tA[:], tA[:], ws_pm[:, :, 27 + j], Alu.add)
                    nc.scalar.activation(outt[:, :, j], tA[:], Act.Sigmoid)

                nc.sync.dma_start(out=out_d[:].rearrange("(p c) d -> p c d", p=P),
                                  in_=outt[:])
    nc.compile()
    return nc


def _pack_weights(lw1, lb1, lw2, lb2, lw3, lb3, ww1, wb1, ww2, wb2, ww3, wb3):
    W1c = np.zeros((W2, W2), np.float32)
    for l in range(L):
        W1c[4 * l + 0, 0:W] = lw1[2 * l + 0]
        W1c[4 * l + 1, 0:W] = lw1[2 * l + 1]
        W1c[4 * l + 2, W:W2] = ww1[2 * l + 0]
        W1c[4 * l + 3, W:W2] = ww1[2 * l + 1]
    W2c = np.zeros((W2, W2), np.float32)
    W2c[0:W, 0:W] = lw2
    W2c[W:W2, W:W2] = ww2
    W3c = np.zeros((W2, NO3), np.float32)
    W3c[0:W, 0:16] = lw3
    W3c[W:W2, 16:NO3] = ww3
    mlpw = np.concatenate([W1c, W2c, W3c], axis=1).astype(bfloat16)
    mlpb = np.zeros((W2, 3), np.float32)
    mlpb[:, 0] = np.concatenate([lb1, wb1])
    mlpb[:, 1] = np.concatenate([lb2, wb2])
    mlpb[:NO3, 2] = np.concatenate([lb3, wb3])
    return mlpw, mlpb


def _make_in_maps(inputs):
    xs = np.asarray(inputs["xs"], dtype=np.float32)
    ds = np.asarray(inputs["ds"], dtype=np.float32)
    emb_il = np.concatenate(
        [np.asarray(inputs["emb_x"], np.float32).reshape(L * T, F),
         np.asarray(inputs["emb_w"], np.float32).reshape(L * T, F)], axis=1)
    embl = np.ascontiguousarray(emb_il).reshape(L, NPAGE, 64)
    mlpw, mlpb = _pack_weights(
        *[np.asarray(inputs[k], np.float32) for k in
          ("lw1", "lb1", "lw2", "lb2", "lw3", "lb3",
           "ww1", "wb1", "ww2", "wb2", "ww3", "wb3")])
    in_maps = []
    for r in range(NCORES):
        sl = slice(r * NLOC, (r + 1) * NLOC)
        m = {"xs": np.ascontiguousarray(xs[sl]),
             "ds": np.ascontiguousarray(ds[sl]),
             "mlpw": mlpw, "mlpb": mlpb}
        for l in range(L):
            m[f"embl{l}"] = embl[l]
        in_maps.append(m)
    return in_maps


def kernel(xs, ds, emb_x, emb_w, lw1, lb1, lw2, lb2, lw3, lb3,
           ww1, wb1, ww2, wb2, ww3, wb3):
    in_maps = _make_in_maps(dict(
        xs=xs, ds=ds, emb_x=emb_x, emb_w=emb_w,
        lw1=lw1, lb1=lb1, lw2=lw2, lb2=lb2, lw3=lw3, lb3=lb3,
        ww1=ww1, wb1=wb1, ww2=ww2, wb2=wb2, ww3=ww3, wb3=wb3))

    if "nc" not in _NC_CACHE:
        _NC_CACHE["nc"] = build_nc()
    nc = _NC_CACHE["nc"]

    res = run_bass_kernel_spmd(nc, in_maps, list(range(NCORES)),
                               trace=_RUN_OPTS.get("trace", False),
                               trace_cores=_RUN_OPTS.get("trace_cores"))
    _LAST["res"] = res
    return np.concatenate([res.results[r]["outc"] for r in range(NCORES)], axis=0)
